# revision 16
# baseline (speedup 1.0000x reference)
"""Trainium2 Bass kernel for DKD-style keypoint detection (nms_detection).

Pipeline (matches the jax reference exactly, including argsort tie semantics):
  1. Border-zero the 2048x2048 score map (host, cheap).
  2. Device (8 NeuronCores, H-sharded, 256 rows each): 4x4 tile max-pool —
     the memory-bound scan of the score map. Each core reads its 2 MB shard
     and emits 64x512 per-tile maxima.
  3. Host: exact stable-argsort top-1000 over the 512x512 tile maxima
     (262144 floats — tiny), recover in-tile argmax for the 1000 winners
     from the zeroed score map, gather + L2-normalize descriptors at the
     1000 keypoints.
"""

import os
import sys
from contextlib import ExitStack

import numpy as np

if os.path.isdir("/opt/trn_rl_repo") and "/opt/trn_rl_repo" not in sys.path:
    sys.path.insert(0, "/opt/trn_rl_repo")

import concourse.bass as bass
import concourse.mybir as mybir
from concourse.bass_utils import run_bass_kernel_spmd

RADIUS = 2
TOP_K = 1000
K = 4                 # pool kernel; tiles are K x K
H = W = 2048
N_CORES = 8
ROWS = H // N_CORES   # 256 image rows per core
TRL = ROWS // K       # 64 tile-rows per core
TC = W // K           # 512 tile-cols
HTC = TC // 2         # 256 tile-cols per partition-half

_NC = None
LAST_RESULTS = None   # BassKernelResults of the most recent device run


BLK = 128             # image rows per block (one SBUF load)
NB = ROWS // BLK      # 2 blocks per core
TRB = BLK // K        # 32 tile-rows per block
HW_ = W // 2          # 1024 cols per half


def _build_nc(with_drains: bool = False):
    """Per-core program: [256, 2048] f32 scores shard -> [2, 128, 512] col-maxima.

    Two contiguous [128, 2048] block loads (8 KB/partition descriptors, full
    SDMA fan-out, FIFO order so block 0 lands halfway through the DMA phase).
    Per block on DVE (forward-streaming ops, no drains needed on HW;
    `with_drains` builds the sim-checkable variant):
      m1 = pairwise col max (stride-2 TT)   [128, 1024]
      h  = pairwise col max                 [128, 512]   (4:1 col reduce)
    h[b][p, tc] = max over cols 4tc..4tc+3 of image row b*128+p. The cheap
    cross-row (partition) 4:1 max runs on the host.
    """
    nc = bass.Bass(target_bir_lowering=False,
                   detect_race_conditions=with_drains,
                   enable_partition_id=False)
    scores = nc.declare_dram_parameter(
        "scores", [ROWS, W], mybir.dt.float32, isOutput=False
    )
    out = nc.declare_dram_parameter(
        "out", [NB, 128, TC], mybir.dt.float32, isOutput=True
    )

    es = ExitStack()
    with (
        es,
        nc.sbuf_tensor("sb0", [128, W], mybir.dt.float32) as sb0,
        nc.sbuf_tensor("sb1", [128, W], mybir.dt.float32) as sb1,
        nc.sbuf_tensor("m1b", [128, W // 2], mybir.dt.float32) as m1b,
        nc.sbuf_tensor("h0", [128, TC], mybir.dt.float32) as h0,
        nc.sbuf_tensor("h1", [128, TC], mybir.dt.float32) as h1,
        nc.Block(no_gpsimd_drain=True) as block,
    ):
        sbs = [sb0, sb1]
        hs = [h0, h1]
        in_sems = [es.enter_context(nc.semaphore(f"in_sem{b}")) for b in range(NB)]
        out_sem = es.enter_context(nc.semaphore("out_sem"))
        v_sem = es.enter_context(nc.semaphore("v_sem"))

        def pair(ap):
            v = ap.rearrange("p (x two) -> p two x", two=2)
            return v[:, 0, :], v[:, 1, :]

        @block.sync
        def _(sync):
            for b in range(NB):
                sync.dma_start(
                    sbs[b][:, :], scores[b * BLK:(b + 1) * BLK, :]
                ).then_inc(in_sems[b], 16)
            HTC2 = TC // 2
            for b in range(NB):
                sync.wait_ge(v_sem, 2 * b + 1)
                sync.dma_start(
                    out[b, :, :HTC2], hs[b][:, :HTC2]).then_inc(out_sem, 16)

        @block.scalar
        def _(scalar):
            HTC2 = TC // 2
            for b in range(NB):
                scalar.wait_ge(v_sem, 2 * b + 2)
                scalar.dma_start(
                    out[b, :, HTC2:], hs[b][:, HTC2:]).then_inc(out_sem, 16)

        @block.vector
        def _(vector):
            def dr():
                if with_drains:
                    vector.drain()
            HTC2 = TC // 2
            for b in range(NB):
                vector.wait_ge(in_sems[b], 16)
                e, o = pair(sbs[b][:, :])
                vector.tensor_max(m1b[:, :], e, o)
                dr()
                for half in range(2):
                    e, o = pair(m1b[:, half * (W // 4):(half + 1) * (W // 4)])
                    vector.tensor_max(
                        hs[b][:, half * HTC2:(half + 1) * HTC2], e, o)
                    vector.drain().then_inc(v_sem, 1)

    return nc


def _get_nc():
    global _NC
    if _NC is None:
        _NC = _build_nc()
    return _NC


def _device_tile_max(s: np.ndarray, trace: bool = False) -> np.ndarray:
    """s: [2048, 2048] border-zeroed scores -> [512, 512] per-tile maxima."""
    global LAST_RESULTS
    in_maps = [
        {"scores": np.ascontiguousarray(s[c * ROWS:(c + 1) * ROWS])}
        for c in range(N_CORES)
    ]
    res = run_bass_kernel_spmd(
        _get_nc(), in_maps, core_ids=list(range(N_CORES)), trace=trace
    )
    LAST_RESULTS = res
    vals = np.empty((TC, TC), dtype=np.float32)
    for c in range(N_CORES):
        o = np.asarray(res.results[c]["out"])  # [NB, 128, 512] col-maxima
        # vertical 4:1 on host: rows 4q..4q+3 -> tile-row q
        v = o.reshape(NB * 128 // K, K, TC).max(axis=1)  # [64, 512]
        vals[c * TRL:(c + 1) * TRL] = v
    return vals


def kernel(scores_map: np.ndarray, descriptor_map: np.ndarray, _trace: bool = False):
    s = np.array(np.asarray(scores_map)[0, 0], dtype=np.float32, copy=True)
    s[: RADIUS + 1, :] = 0.0
    s[:, : RADIUS + 1] = 0.0
    s[H - RADIUS:, :] = 0.0
    s[:, W - RADIUS:] = 0.0

    vals = _device_tile_max(s, trace=_trace)  # [512, 512]

    # Exact replication of jnp.argsort(vals.ravel())[-TOP_K:] (stable sort).
    flat = np.argsort(vals.ravel(), kind="stable")[-TOP_K:]
    tr = flat // TC
    tc = flat % TC

    # Recover first-occurrence in-tile argmax for the 1000 winners only.
    s4 = s.reshape(TC, K, TC, K)
    sel = s4[tr[:, None, None], np.arange(K)[None, :, None], tc[:, None, None],
             np.arange(K)[None, None, :]]        # [1000, K, K]
    arg = np.argmax(sel.reshape(-1, K * K), axis=1)

    rows = (tr * K + arg // K).astype(np.int64)
    cols = (tc * K + arg % K).astype(np.int64)
    kpts = np.stack([cols, rows], axis=1).astype(np.int32)   # [N, 2] (x, y)
    kptscores = vals.ravel()[flat].astype(np.float32)

    d = np.asarray(descriptor_map)[0][:, rows, cols].astype(np.float32)  # [64, N]
    d = d / np.linalg.norm(d, axis=0, keepdims=True)
    descriptors = np.ascontiguousarray(d.T)[:, :, None]      # [N, 64, 1]

    return kpts, descriptors, kptscores


# revision 18
# speedup vs baseline: 1.0044x; 1.0044x over previous
"""Trainium2 Bass kernel for DKD-style keypoint detection (nms_detection).

Pipeline (matches the jax reference exactly, including argsort tie semantics):
  1. Border-zero the 2048x2048 score map (host, cheap).
  2. Device (8 NeuronCores, H-sharded, 256 rows each): the memory-bound scan
     of the score map. Each core streams its 2 MB shard through DVE pairwise
     maxes (4:1 column reduce) and ships [2, 128, 512] column-maxima back,
     with DMA / compute / writeback pipelined across two row blocks and the
     two HWDGE queues.
  3. Host: finish the tiny 4:1 cross-row max (-> 512x512 tile maxima), exact
     stable-argsort top-1000 (262144 floats), recover first-occurrence
     in-tile argmax for the 1000 winners from the zeroed score map, gather +
     L2-normalize descriptors at the 1000 keypoints.
"""

import os
import sys
from contextlib import ExitStack

import numpy as np

if os.path.isdir("/opt/trn_rl_repo") and "/opt/trn_rl_repo" not in sys.path:
    sys.path.insert(0, "/opt/trn_rl_repo")

import concourse.bass as bass
import concourse.mybir as mybir
from concourse.bass_utils import run_bass_kernel_spmd

RADIUS = 2
TOP_K = 1000
K = 4                 # pool kernel; tiles are K x K
H = W = 2048
N_CORES = 8
ROWS = H // N_CORES   # 256 image rows per core
TRL = ROWS // K       # 64 tile-rows per core
TC = W // K           # 512 tile-cols
HTC = TC // 2         # 256 tile-cols per partition-half

_NC = None
LAST_RESULTS = None   # BassKernelResults of the most recent device run


BLK = 128             # image rows per block (one SBUF load)
NB = ROWS // BLK      # 2 blocks per core
TRB = BLK // K        # 32 tile-rows per block
HW_ = W // 2          # 1024 cols per half


def _build_nc(with_drains: bool = False):
    """Per-core program: [256, 2048] f32 scores shard -> [2, 128, 512] col-maxima.

    Two contiguous [128, 2048] block loads (8 KB/partition descriptors, full
    SDMA fan-out, FIFO order so block 0 lands halfway through the DMA phase).
    Per block on DVE (forward-streaming ops, no drains needed on HW;
    `with_drains` builds the sim-checkable variant):
      m1 = pairwise col max (stride-2 TT)   [128, 1024]
      h  = pairwise col max                 [128, 512]   (4:1 col reduce)
    h[b][p, tc] = max over cols 4tc..4tc+3 of image row b*128+p. The cheap
    cross-row (partition) 4:1 max runs on the host.
    """
    nc = bass.Bass(target_bir_lowering=False,
                   detect_race_conditions=with_drains,
                   enable_partition_id=False)
    scores = nc.declare_dram_parameter(
        "scores", [ROWS, W], mybir.dt.float32, isOutput=False
    )
    out = nc.declare_dram_parameter(
        "out", [NB, 128, TC], mybir.dt.float32, isOutput=True
    )

    es = ExitStack()
    with (
        es,
        nc.sbuf_tensor("sb0", [128, W], mybir.dt.float32) as sb0,
        nc.sbuf_tensor("sb1", [128, W], mybir.dt.float32) as sb1,
        nc.sbuf_tensor("m1b", [128, W // 2], mybir.dt.float32) as m1b,
        nc.sbuf_tensor("h0", [128, TC], mybir.dt.float32) as h0,
        nc.sbuf_tensor("h1", [128, TC], mybir.dt.float32) as h1,
        nc.Block(no_gpsimd_drain=True) as block,
    ):
        sbs = [sb0, sb1]
        hs = [h0, h1]
        in_sems = [es.enter_context(nc.semaphore(f"in_sem{b}")) for b in range(NB)]
        out_sem = es.enter_context(nc.semaphore("out_sem"))
        v_sem = es.enter_context(nc.semaphore("v_sem"))

        def pair(ap):
            v = ap.rearrange("p (x two) -> p two x", two=2)
            return v[:, 0, :], v[:, 1, :]

        @block.sync
        def _(sync):
            for b in range(NB):
                sync.dma_start(
                    sbs[b][:, :], scores[b * BLK:(b + 1) * BLK, :]
                ).then_inc(in_sems[b], 16)
            HTC2 = TC // 2
            for b in range(NB):
                sync.wait_ge(v_sem, 2 * b + 1)
                sync.dma_start(
                    out[b, :, :HTC2], hs[b][:, :HTC2]).then_inc(out_sem, 16)

        @block.scalar
        def _(scalar):
            HTC2 = TC // 2
            for b in range(NB):
                scalar.wait_ge(v_sem, 2 * b + 2)
                scalar.dma_start(
                    out[b, :, HTC2:], hs[b][:, HTC2:]).then_inc(out_sem, 16)

        @block.vector
        def _(vector):
            def dr():
                if with_drains:
                    vector.drain()
            HTC2 = TC // 2
            for b in range(NB):
                vector.wait_ge(in_sems[b], 16)
                e, o = pair(sbs[b][:, :])
                vector.tensor_max(m1b[:, :], e, o)
                dr()
                for half in range(2):
                    e, o = pair(m1b[:, half * (W // 4):(half + 1) * (W // 4)])
                    vector.tensor_max(
                        hs[b][:, half * HTC2:(half + 1) * HTC2], e, o)
                    vector.drain().then_inc(v_sem, 1)

    return nc


def _get_nc():
    global _NC
    if _NC is None:
        _NC = _build_nc()
    return _NC


def _device_tile_max(s: np.ndarray, trace: bool = False) -> np.ndarray:
    """s: [2048, 2048] border-zeroed scores -> [512, 512] per-tile maxima."""
    global LAST_RESULTS
    in_maps = [
        {"scores": np.ascontiguousarray(s[c * ROWS:(c + 1) * ROWS])}
        for c in range(N_CORES)
    ]
    try:
        res = run_bass_kernel_spmd(
            _get_nc(), in_maps, core_ids=list(range(N_CORES)), trace=trace
        )
    except Exception:
        # one retry for transient device/transport hiccups
        res = run_bass_kernel_spmd(
            _get_nc(), in_maps, core_ids=list(range(N_CORES)), trace=trace
        )
    LAST_RESULTS = res
    vals = np.empty((TC, TC), dtype=np.float32)
    for c in range(N_CORES):
        o = np.asarray(res.results[c]["out"])  # [NB, 128, 512] col-maxima
        # vertical 4:1 on host: rows 4q..4q+3 -> tile-row q
        v = o.reshape(NB * 128 // K, K, TC).max(axis=1)  # [64, 512]
        vals[c * TRL:(c + 1) * TRL] = v
    return vals


def kernel(scores_map: np.ndarray, descriptor_map: np.ndarray, _trace: bool = False):
    s = np.array(np.asarray(scores_map)[0, 0], dtype=np.float32, copy=True)
    s[: RADIUS + 1, :] = 0.0
    s[:, : RADIUS + 1] = 0.0
    s[H - RADIUS:, :] = 0.0
    s[:, W - RADIUS:] = 0.0

    vals = _device_tile_max(s, trace=_trace)  # [512, 512]

    # Exact replication of jnp.argsort(vals.ravel())[-TOP_K:] (stable sort).
    flat = np.argsort(vals.ravel(), kind="stable")[-TOP_K:]
    tr = flat // TC
    tc = flat % TC

    # Recover first-occurrence in-tile argmax for the 1000 winners only.
    s4 = s.reshape(TC, K, TC, K)
    sel = s4[tr[:, None, None], np.arange(K)[None, :, None], tc[:, None, None],
             np.arange(K)[None, None, :]]        # [1000, K, K]
    arg = np.argmax(sel.reshape(-1, K * K), axis=1)

    rows = (tr * K + arg // K).astype(np.int64)
    cols = (tc * K + arg % K).astype(np.int64)
    kpts = np.stack([cols, rows], axis=1).astype(np.int32)   # [N, 2] (x, y)
    kptscores = vals.ravel()[flat].astype(np.float32)

    d = np.asarray(descriptor_map)[0][:, rows, cols].astype(np.float32)  # [64, N]
    d = d / np.linalg.norm(d, axis=0, keepdims=True)
    descriptors = np.ascontiguousarray(d.T)[:, :, None]      # [N, 64, 1]

    return kpts, descriptors, kptscores
